# revision 1
# baseline (speedup 1.0000x reference)
"""Co-attention fusion kernel for 8 TRN2 NeuronCores.

Strategy (row-parallel flash attention per the sharding hint):
- Shard rows (N=8192) of image/tabular features across 8 cores (1024 each).
- Each core computes its local K^T / V projection shards, AllGathers them
  (K^T in fp32[r], V in bf16), then computes its 1024 query rows against the
  full gathered keys/values, plus the output projection for its row shard.

Numerics: the softmax logits here have std ~13 (range +-87), so the Q/K
projections and QK^T run in float32r (full-rate reduced-precision fp32 on the
PE: ~0.009 max logit error vs 0.14 for bf16). V, A@V and the output
projection run in bf16. Softmax uses a fixed shift M=96 instead of a row max
(exp(s-96) cannot overflow for logits < 184 and keeps all weights within
bf16/fp32 range for row maxima >= ~16; actual row maxima are 44..87), which
removes the max-reduction from the critical path entirely.
"""

import os
import numpy as np
import ml_dtypes

import concourse.bacc as bacc
import concourse.mybir as mybir
import concourse.tile as tile
from concourse.bass_utils import run_bass_kernel_spmd

N = 8192
D = 1024
NCORES = 8
SH = N // NCORES  # 1024 rows per core
NCH = D // 128  # 8 contraction chunks
M_SHIFT = 96.0  # softmax shift (see module docstring)

f32 = mybir.dt.float32
f32r = mybir.dt.float32r
bf16 = mybir.dt.bfloat16

HALF = 4  # q-subblocks (128 rows) per attention phase


def build_nc():
    nc = bacc.Bacc(trn_type="TRN2", num_devices=NCORES)

    # ---- parameters ----
    xTi = nc.declare_dram_parameter("xTi", [D, SH], f32, isOutput=False)
    xTt = nc.declare_dram_parameter("xTt", [D, SH], f32, isOutput=False)
    Ws = {
        name: nc.declare_dram_parameter(name, [D, D], f32, isOutput=False)
        for name in ["Wqi", "Wkt", "Wvt", "Wqt", "Wki", "Wvi"]
    }
    Wo16 = nc.declare_dram_parameter("Wo16", [2 * D, 2 * D], bf16, isOutput=False)
    Bs = {
        name: nc.declare_dram_parameter(name, [1, D], f32, isOutput=False)
        for name in ["bqi", "bkt", "bvt", "bqt", "bki", "bvi"]
    }
    bo16 = nc.declare_dram_parameter("bo16", [1, 2 * D], bf16, isOutput=False)
    ident = nc.declare_dram_parameter("ident", [128, 128], bf16, isOutput=False)
    ones32 = nc.declare_dram_parameter("ones32", [1, 512], f32, isOutput=False)
    ones16 = nc.declare_dram_parameter("ones16", [1, 512], bf16, isOutput=False)
    out = nc.declare_dram_parameter("out", [SH, 2 * D], f32, isOutput=True)

    # ---- internal DRAM ----
    # Per-branch AllGather bounces: K^T [out_d, local keys] f32, V natural
    # [local key, d] bf16. Shared outputs = fast HBM-HBM collective path.
    bk_in = [nc.dram_tensor(f"bk_in{i}", [D, SH], f32) for i in range(2)]
    bv_in = [nc.dram_tensor(f"bv_in{i}", [SH, D], bf16) for i in range(2)]
    gath_k = [
        nc.dram_tensor(f"gath_k{i}", [N, SH], f32, addr_space="Shared")
        for i in range(2)
    ]
    gath_v = [
        nc.dram_tensor(f"gath_v{i}", [N, D], bf16, addr_space="Shared")
        for i in range(2)
    ]
    qT_dram = [nc.dram_tensor(f"qT{b}", [D, SH], f32) for b in range(2)]

    def ch(handle2d, colslice=None):
        """DRAM [R, C] -> [128, R/128, C'] AP (partition=row%128, chunked)."""
        ap = handle2d[:, :] if colslice is None else handle2d[:, colslice]
        return ap.rearrange("(c p) x -> p c x", p=128)

    with tile.TileContext(nc) as tc:
        # ============== stage 1: projections + AllGather ==============
        with (
            tc.tile_pool(name="s1", bufs=1) as s1,
            tc.tile_pool(name="s1w", bufs=2) as s1w,
            tc.tile_pool(name="s1s", bufs=4) as s1s,
            tc.tile_pool(name="ps1", bufs=4, space="PSUM") as ps1,
        ):
            xti = s1.tile([128, NCH, SH], f32r, tag="xti")
            xtt = s1.tile([128, NCH, SH], f32r, tag="xtt")
            nc.sync.dma_start(out=xti[:], in_=ch(xTi).bitcast(f32r))
            nc.sync.dma_start(out=xtt[:], in_=ch(xTt).bitcast(f32r))
            ones32_sb = s1.tile([1, 512], f32r, tag="ones32")
            nc.sync.dma_start(out=ones32_sb[:], in_=ones32[:, :].bitcast(f32r))
            brow = {}
            for bn in ("bvt", "bvi"):
                brow[bn] = s1.tile([1, D], f32r, tag=bn, name="brow_" + bn)
                nc.sync.dma_start(out=brow[bn][:], in_=Bs[bn][:, :].bitcast(f32r))
            bcol = {}
            for bn in ("bkt", "bki", "bqi", "bqt"):
                bcol[bn] = s1.tile([128, NCH], f32, tag=bn, name="bcol_" + bn)
                nc.sync.dma_start(
                    out=bcol[bn][:], in_=Bs[bn][0, :].rearrange("(c p) -> p c", p=128)
                )

            def load_w(wname):
                w = s1w.tile([128, NCH, D], f32r, tag="w")
                nc.sync.dma_start(out=w[:], in_=ch(Ws[wname]).bitcast(f32r))
                return w

            def proj_T(wname, bname, xt, dst_dram, dst_col0):
                """q^T/k^T projection: out[d_out, rows] blocks -> DRAM."""
                w = load_w(wname)
                for od in range(NCH):
                    for rt in range(2):
                        ps = ps1.tile([128, 512], f32, tag="pp")
                        for c in range(NCH):
                            nc.tensor.matmul(
                                ps[:],
                                w[:, c, od * 128 : (od + 1) * 128],
                                xt[:, c, rt * 512 : (rt + 1) * 512],
                                start=(c == 0),
                                stop=(c == NCH - 1),
                            )
                        stg = s1s.tile([128, 512], f32r, tag="stg")
                        nc.vector.tensor_scalar_add(
                            stg[:], ps[:], bcol[bname][:, od : od + 1]
                        )
                        nc.sync.dma_start(
                            out=dst_dram[
                                od * 128 : (od + 1) * 128,
                                dst_col0 + rt * 512 : dst_col0 + (rt + 1) * 512,
                            ].bitcast(f32r),
                            in_=stg[:],
                        )

            def proj_V(wname, bname, xt, dst_col0, dst_bv):
                """v projection, natural [rows, d_out] -> bf16 bounce."""
                w = load_w(wname)
                for rt in range(NCH):
                    for ot in range(2):
                        ps = ps1.tile([128, 512], f32, tag="pp")
                        for c in range(NCH):
                            nc.tensor.matmul(
                                ps[:],
                                xt[:, c, rt * 128 : (rt + 1) * 128],
                                w[:, c, ot * 512 : (ot + 1) * 512],
                                start=(c == 0),
                                stop=False,
                            )
                        nc.tensor.matmul(
                            ps[:],
                            ones32_sb[0:1, 0:128],
                            brow[bname][0:1, ot * 512 : (ot + 1) * 512],
                            start=False,
                            stop=True,
                        )
                        stg = s1s.tile([128, 512], bf16, tag="vstg")
                        nc.vector.tensor_copy(stg[:], ps[:])
                        nc.sync.dma_start(
                            out=dst_bv[
                                rt * 128 : (rt + 1) * 128,
                                dst_col0 + ot * 512 : dst_col0 + (ot + 1) * 512,
                            ],
                            in_=stg[:],
                        )

            # K/V first, one AllGather right after each projection so the
            # collective queue drains while later projections run on the PE.
            rg = [list(range(NCORES))]

            def ag(src_t, dst_t):
                nc.gpsimd.collective_compute(
                    "AllGather",
                    mybir.AluOpType.bypass,
                    replica_groups=rg,
                    ins=[src_t.ap().opt()],
                    outs=[dst_t.ap().opt()],
                )

            proj_T("Wkt", "bkt", xtt, bk_in[0], 0)
            ag(bk_in[0], gath_k[0])
            proj_V("Wvt", "bvt", xtt, 0, bv_in[0])
            ag(bv_in[0], gath_v[0])
            proj_T("Wki", "bki", xti, bk_in[1], 0)
            ag(bk_in[1], gath_k[1])
            proj_V("Wvi", "bvi", xti, 0, bv_in[1])
            ag(bv_in[1], gath_v[1])

            # q projections overlap the AllGathers
            proj_T("Wqi", "bqi", xti, qT_dram[0], 0)
            proj_T("Wqt", "bqt", xtt, qT_dram[1], 0)

        # ============== stage 3: attention + output projection ==============
        with (
            tc.tile_pool(name="s3", bufs=1) as s3,
            tc.tile_pool(name="s3k", bufs=3) as s3k,
            tc.tile_pool(name="s3v", bufs=3) as s3v,
            tc.tile_pool(name="s3at", bufs=2) as s3at,
            tc.tile_pool(name="s3o", bufs=1) as s3o,
            tc.tile_pool(name="ps3", bufs=2, space="PSUM") as ps3,
            tc.tile_pool(name="psav", bufs=1, space="PSUM") as psav,
        ):
            ident_sb = s3.tile([128, 128], bf16, tag="ident")
            nc.sync.dma_start(out=ident_sb[:], in_=ident[:, :])
            ones16_sb = s3.tile([1, 512], bf16, tag="ones16")
            nc.sync.dma_start(out=ones16_sb[:], in_=ones16[:, :])
            bo_sb = s3.tile([1, 2 * D], bf16, tag="bo")
            nc.sync.dma_start(out=bo_sb[:], in_=bo16[:, :])
            negm = s3.tile([128, 1], f32, tag="negm")
            nc.vector.memset(negm[:], -M_SHIFT)

            A = s3.tile([128, HALF, N], bf16, tag="A")
            lsum = s3.tile([128, HALF, 16], f32, tag="lsum")
            ltot = s3.tile([128, HALF], f32, tag="ltot")
            linv = s3.tile([128, HALF], f32, tag="linv")
            fused = s3.tile([128, HALF, 2 * D], bf16, tag="fused")

            for h in range(2):
                for b in range(2):
                    # reload this branch's q^T
                    qt = s3.tile([128, NCH, SH], f32r, tag="qt")
                    dmae = nc.scalar if b == 0 else nc.sync
                    dmae.dma_start(out=qt[:], in_=ch(qT_dram[b]).bitcast(f32r))

                    # ---- S phase: A[qs] = exp(q_blk @ K^T - M), l = row sums
                    for kt in range(16):
                        r, j0 = kt // 2, (kt % 2) * 512
                        kta = s3k.tile([128, 4, 512], f32r, tag="kta")
                        ktb = s3k.tile([128, 4, 512], f32r, tag="ktb")
                        dmae.dma_start(
                            out=kta[:],
                            in_=gath_k[b][
                                r * SH : r * SH + 512, j0 : j0 + 512
                            ]
                            .rearrange("(c p) k -> p c k", p=128)
                            .bitcast(f32r),
                        )
                        dmae.dma_start(
                            out=ktb[:],
                            in_=gath_k[b][
                                r * SH + 512 : r * SH + 1024, j0 : j0 + 512
                            ]
                            .rearrange("(c p) k -> p c k", p=128)
                            .bitcast(f32r),
                        )
                        for q in range(HALF):
                            qg = h * HALF + q
                            ps = ps3.tile([128, 512], f32, tag="s")
                            for c in range(NCH):
                                src = kta if c < 4 else ktb
                                nc.tensor.matmul(
                                    ps[:],
                                    qt[:, c, qg * 128 : (qg + 1) * 128],
                                    src[:, c % 4, :],
                                    start=(c == 0),
                                    stop=(c == NCH - 1),
                                )
                            nc.scalar.activation(
                                A[:, q, kt * 512 : (kt + 1) * 512],
                                ps[:],
                                mybir.ActivationFunctionType.Exp,
                                bias=negm[:, 0:1],
                                scale=1.0,
                                accum_out=lsum[:, q, kt : kt + 1],
                            )

                    # ---- softmax normalization factors (applied at AV output)
                    for q in range(HALF):
                        nc.vector.tensor_reduce(
                            ltot[:, q : q + 1],
                            lsum[:, q, :],
                            axis=mybir.AxisListType.X,
                            op=mybir.AluOpType.add,
                        )
                        nc.vector.reciprocal(linv[:, q : q + 1], ltot[:, q : q + 1])

                    # ---- AV phase: attended[qs] = A[qs] @ V  (qs pairs)
                    fofs = D if b == 0 else 0  # b0 -> attended_tabular (cols D:2D)
                    for pair in range(HALF // 2):
                        avp = [
                            [psav.tile([128, 512], f32, tag=f"av{i}{dh}", name=f"av{i}{dh}") for dh in range(2)]
                            for i in range(2)
                        ]
                        for kc in range(64):
                            vt = s3v.tile([128, D], bf16, tag="vt")
                            dmae.dma_start(
                                out=vt[:],
                                in_=gath_v[b][kc * 128 : (kc + 1) * 128, :],
                            )
                            for i in range(2):
                                q = pair * 2 + i
                                pt = ps3.tile([128, 128], bf16, tag="t")
                                nc.tensor.transpose(
                                    pt[:], A[:, q, kc * 128 : (kc + 1) * 128], ident_sb[:]
                                )
                                at = s3at.tile([128, 128], bf16, tag="at")
                                nc.vector.tensor_copy(at[:], pt[:])
                                for dh in range(2):
                                    nc.tensor.matmul(
                                        avp[i][dh][:],
                                        at[:],
                                        vt[:, dh * 512 : (dh + 1) * 512],
                                        start=(kc == 0),
                                        stop=(kc == 63),
                                    )
                        for i in range(2):
                            q = pair * 2 + i
                            for dh in range(2):
                                nc.vector.tensor_scalar_mul(
                                    fused[:, q, fofs + dh * 512 : fofs + (dh + 1) * 512],
                                    avp[i][dh][:],
                                    linv[:, q : q + 1],
                                )

                # ---- output projection for this half (512 q rows)
                fts = []
                for q in range(HALF):
                    ft = s3.tile([128, 16, 128], bf16, tag=f"ft{q}")
                    for f in range(16):
                        pt = ps3.tile([128, 128], bf16, tag="t")
                        nc.tensor.transpose(
                            pt[:], fused[:, q, f * 128 : (f + 1) * 128], ident_sb[:]
                        )
                        nc.vector.tensor_copy(ft[:, f, :], pt[:])
                    fts.append(ft)
                for od in range(4):
                    wo = s3.tile([128, 16, 512], bf16, tag="wo")
                    nc.scalar.dma_start(
                        out=wo[:],
                        in_=Wo16[:, od * 512 : (od + 1) * 512].rearrange(
                            "(c p) o -> p c o", p=128
                        ),
                    )
                    for q in range(HALF):
                        qg = h * HALF + q
                        ps = ps3.tile([128, 512], f32, tag="s")
                        for f in range(16):
                            nc.tensor.matmul(
                                ps[:], fts[q][:, f, :], wo[:, f, :],
                                start=(f == 0), stop=False,
                            )
                        nc.tensor.matmul(
                            ps[:],
                            ones16_sb[0:1, 0:128],
                            bo_sb[0:1, od * 512 : (od + 1) * 512],
                            start=False,
                            stop=True,
                        )
                        ost = s3o.tile([128, 512], f32, tag="ost")
                        nc.vector.tensor_copy(ost[:], ps[:])
                        nc.sync.dma_start(
                            out=out[qg * 128 : (qg + 1) * 128, od * 512 : (od + 1) * 512],
                            in_=ost[:],
                        )

    nc.compile()
    return nc


_CACHE: dict = {}


def kernel(
    image_features, tabular_features,
    Wqi, bqi, Wkt, bkt, Wvt, bvt,
    Wqt, bqt, Wki, bki, Wvi, bvi,
    Wo, bo,
) -> np.ndarray:
    if "nc" not in _CACHE:
        _CACHE["nc"] = build_nc()
    nc = _CACHE["nc"]

    img = np.asarray(image_features, np.float32)
    tab = np.asarray(tabular_features, np.float32)
    shared = {
        "Wqi": np.asarray(Wqi, np.float32), "Wkt": np.asarray(Wkt, np.float32),
        "Wvt": np.asarray(Wvt, np.float32), "Wqt": np.asarray(Wqt, np.float32),
        "Wki": np.asarray(Wki, np.float32), "Wvi": np.asarray(Wvi, np.float32),
        "Wo16": np.asarray(Wo).astype(ml_dtypes.bfloat16),
        "bqi": np.asarray(bqi, np.float32).reshape(1, D),
        "bkt": np.asarray(bkt, np.float32).reshape(1, D),
        "bvt": np.asarray(bvt, np.float32).reshape(1, D),
        "bqt": np.asarray(bqt, np.float32).reshape(1, D),
        "bki": np.asarray(bki, np.float32).reshape(1, D),
        "bvi": np.asarray(bvi, np.float32).reshape(1, D),
        "bo16": np.asarray(bo).astype(ml_dtypes.bfloat16).reshape(1, 2 * D),
        "ident": np.eye(128, dtype=ml_dtypes.bfloat16),
        "ones32": np.ones((1, 512), np.float32),
        "ones16": np.ones((1, 512), ml_dtypes.bfloat16),
    }
    in_maps = []
    for c in range(NCORES):
        m = dict(shared)
        m["xTi"] = np.ascontiguousarray(img[c * SH : (c + 1) * SH, :].T)
        m["xTt"] = np.ascontiguousarray(tab[c * SH : (c + 1) * SH, :].T)
        in_maps.append(m)

    trace = bool(int(os.environ.get("KERNEL_TRACE", "0")))
    res = run_bass_kernel_spmd(
        nc, in_maps, core_ids=list(range(NCORES)), trace=trace
    )
    _CACHE["last_result"] = res
    return np.concatenate([res.results[c]["out"] for c in range(NCORES)], axis=0)



# revision 15
# speedup vs baseline: 1.3498x; 1.3498x over previous
"""Co-attention fusion kernel for 8 TRN2 NeuronCores.

Strategy (row-parallel flash attention per the sharding hint):
- Shard rows (N=8192) of image/tabular features across 8 cores (1024 each).
- Each core computes its local K^T / V / Q projection shards in f32r,
  casts them to bf16, AllGathers K^T and V (bf16), then computes its 1024
  query rows against the full gathered keys/values plus the output
  projection for its row shard.

Layout trick: S is computed TRANSPOSED (S^T = K @ Q^T, keys on partitions)
so the A@V phase needs no PE transposes at all: attended^T = V^T @ A^T uses
V tiles as the stationary operand directly in natural [key, d] layout, and
attended^T chunks feed the output projection as stationary operands too.
All attention-phase matmuls are bf16 (FWL weight loads overlap streaming).

Numerics: Q/K in bf16 give logit error ~0.14; with the peaked softmax here
(logit std ~13, top-2 gaps ~5) the CPU-simulated end-to-end rel err is
0.0095 vs the 2e-2 gate. Softmax uses a fixed shift M=96 (row maxima are
44..87, so exp(s-96) never overflows and stays in bf16/fp32 range); the
1/l normalization is applied per-query at the *output projection* drain
(per-partition scalar), per branch, fused with the bias add on the DVE.
"""

import os
import numpy as np
import ml_dtypes

import concourse.bacc as bacc
import concourse.mybir as mybir
import concourse.tile as tile
from concourse.bass_utils import run_bass_kernel_spmd

N = 8192
D = 1024
NCORES = 8
SH = N // NCORES  # 1024 rows per core
NCH = D // 128  # 8 contraction chunks
M_SHIFT = 96.0  # softmax shift (see module docstring)

f32 = mybir.dt.float32
f32r = mybir.dt.float32r
bf16 = mybir.dt.bfloat16

OP = mybir.AluOpType
EXP = mybir.ActivationFunctionType.Exp


def build_nc():
    nc = bacc.Bacc(trn_type="TRN2", num_devices=NCORES)

    # ---- parameters ----
    xTi = nc.declare_dram_parameter("xTi", [D, SH], f32, isOutput=False)
    xTt = nc.declare_dram_parameter("xTt", [D, SH], f32, isOutput=False)
    Ws = {
        name: nc.declare_dram_parameter(name, [D, D], f32, isOutput=False)
        for name in ["Wqi", "Wkt", "Wvt", "Wqt", "Wki", "Wvi"]
    }
    Wo16 = nc.declare_dram_parameter("Wo16", [2 * D, 2 * D], bf16, isOutput=False)
    # q/k biases in column layout [dout%128, dout//128]
    Bcol = {
        name: nc.declare_dram_parameter("bc_" + name, [128, NCH], f32, isOutput=False)
        for name in ["bqi", "bkt", "bqt", "bki"]
    }
    # v biases broadcast across partitions
    Brow = {
        name: nc.declare_dram_parameter("br_" + name, [128, D], bf16, isOutput=False)
        for name in ["bvt", "bvi"]
    }
    bob = nc.declare_dram_parameter("bob", [128, 2 * D], bf16, isOutput=False)
    out = nc.declare_dram_parameter("out", [SH, 2 * D], f32, isOutput=True)

    # ---- internal DRAM ----
    bk_in = [nc.dram_tensor(f"bk_in{i}", [D, SH], bf16) for i in range(2)]
    bv_in = [nc.dram_tensor(f"bv_in{i}", [SH, D], bf16) for i in range(2)]
    qTt_d = nc.dram_tensor("qTt_d", [D, SH], bf16)
    l_d = [nc.dram_tensor(f"l_d{i}", [1, SH], f32) for i in range(2)]
    gath_k = [
        nc.dram_tensor(f"gath_k{i}", [N, SH], bf16, addr_space="Shared")
        for i in range(2)
    ]
    gath_v = [
        nc.dram_tensor(f"gath_v{i}", [N, D], bf16, addr_space="Shared")
        for i in range(2)
    ]

    rg = [list(range(NCORES))]

    def ag(src_t, dst_t):
        nc.gpsimd.collective_compute(
            "AllGather",
            OP.bypass,
            replica_groups=rg,
            ins=[src_t.ap().opt()],
            outs=[dst_t.ap().opt()],
        )

    with tile.TileContext(nc) as tc:
        with tc.tile_pool(name="pp", bufs=1) as pp:
            # ---- long-lived tiles ----
            negm = pp.tile([128, 1], f32, tag="negm")
            nc.vector.memset(negm[:], -M_SHIFT)
            ones_col = pp.tile([128, 1], bf16, tag="ones")
            nc.vector.memset(ones_col[:], 1.0)
            qt = pp.tile([128, NCH, SH], bf16, tag="qt")
            fusedT = pp.tile([128, 16, SH], bf16, tag="fusedT")
            bob_sb = pp.tile([128, 2 * D], bf16, tag="bob")
            nc.scalar.dma_start(out=bob_sb[:], in_=bob[:, :])
            linv = [
                pp.tile([128, NCH], f32, tag=f"linv{b}", name=f"linv{b}")
                for b in range(2)
            ]

            # ============== stage 1: projections + AllGather ==============
            with (
                tc.tile_pool(name="s1", bufs=1) as s1,
                tc.tile_pool(name="ps1", bufs=2, space="PSUM") as psp,
            ):
                xti = s1.tile([128, NCH, SH], f32r, tag="xti")
                xtt = s1.tile([128, NCH, SH], f32r, tag="xtt")
                nc.sync.dma_start(
                    out=xti[:],
                    in_=xTi[:, :].rearrange("(c p) x -> p c x", p=128).bitcast(f32r),
                )
                nc.sync.dma_start(
                    out=xtt[:],
                    in_=xTt[:, :].rearrange("(c p) x -> p c x", p=128).bitcast(f32r),
                )
                bcol = {}
                for bn in Bcol:
                    bcol[bn] = s1.tile([128, NCH], f32, tag="bc" + bn, name="bc_" + bn)
                    nc.sync.dma_start(out=bcol[bn][:], in_=Bcol[bn][:, :])
                brow = {}
                for bn in Brow:
                    brow[bn] = s1.tile([128, D], bf16, tag="br" + bn, name="br_" + bn)
                    nc.sync.dma_start(out=brow[bn][:], in_=Brow[bn][:, :])

                def load_w(wname):
                    """W in four 256-col quarters as separate f32r tiles."""
                    wqs = []
                    for wq in range(4):
                        w = s1.tile(
                            [128, NCH, 256], f32r, tag="w", bufs=6,
                            name=f"w_{wname}{wq}",
                        )
                        nc.sync.dma_start(
                            out=w[:],
                            in_=Ws[wname][:, wq * 256 : (wq + 1) * 256]
                            .rearrange("(c p) x -> p c x", p=128)
                            .bitcast(f32r),
                        )
                        wqs.append(w)
                    return wqs

                def proj_T(wname, bname, xt, dst_dram=None, dst_sb=None):
                    """q^T/k^T projection: out[d_out, rows] bf16 blocks."""
                    wqs = load_w(wname)
                    for od in range(NCH):
                        w = wqs[od // 2]
                        odl = od % 2
                        psA = psp.tile([128, 512], f32, tag="mmA")
                        psB = psp.tile([128, 512], f32, tag="mmB")
                        for c in range(NCH):
                            lhsT = w[:, c, odl * 128 : (odl + 1) * 128]
                            nc.tensor.matmul(
                                psA[:], lhsT, xt[:, c, 0:512],
                                start=(c == 0), stop=(c == NCH - 1),
                            )
                            nc.tensor.matmul(
                                psB[:], lhsT, xt[:, c, 512:1024],
                                start=(c == 0), stop=(c == NCH - 1),
                            )
                        for rt, ps in enumerate((psA, psB)):
                            if dst_sb is not None:
                                nc.vector.tensor_scalar_add(
                                    dst_sb[:, od, rt * 512 : (rt + 1) * 512],
                                    ps[:], bcol[bname][:, od : od + 1],
                                )
                            else:
                                stg = s1.tile([128, 512], bf16, tag="stgT", bufs=4)
                                nc.vector.tensor_scalar_add(
                                    stg[:], ps[:], bcol[bname][:, od : od + 1]
                                )
                                nc.gpsimd.dma_start(
                                    out=dst_dram[
                                        od * 128 : (od + 1) * 128,
                                        rt * 512 : (rt + 1) * 512,
                                    ],
                                    in_=stg[:],
                                )

                def proj_V(wname, bname, xt, dst_bv):
                    """v projection, natural [rows, d_out] bf16 blocks.

                    The four 256-wide W quarters accumulate into two PSUM
                    banks (two 256-col column-groups per bank), so only the
                    very first matmul of each bank carries start=True.
                    """
                    wqs = load_w(wname)
                    for rt in range(NCH):
                        psA = psp.tile([128, 512], f32, tag="mmA")
                        psB = psp.tile([128, 512], f32, tag="mmB")
                        for c in range(NCH):
                            lhsT = xt[:, c, rt * 128 : (rt + 1) * 128]
                            for wq in range(4):
                                ps = psA if wq < 2 else psB
                                cs = slice((wq % 2) * 256, (wq % 2) * 256 + 256)
                                nc.tensor.matmul(
                                    ps[:, cs], lhsT, wqs[wq][:, c, :],
                                    start=(c == 0 and wq % 2 == 0),
                                    stop=(c == NCH - 1 and wq % 2 == 1),
                                    skip_group_check=True,
                                )
                        stg = s1.tile([128, D], bf16, tag="stgV", bufs=4)
                        for oh, ps in enumerate((psA, psB)):
                            nc.vector.scalar_tensor_tensor(
                                stg[:, oh * 512 : (oh + 1) * 512],
                                ps[:], 1.0,
                                brow[bname][:, oh * 512 : (oh + 1) * 512],
                                OP.bypass, OP.add,
                            )
                        nc.gpsimd.dma_start(
                            out=dst_bv[rt * 128 : (rt + 1) * 128, :], in_=stg[:]
                        )

                # K/V first; each AllGather queues right after its projection
                # so the collectives drain while the PE keeps projecting.
                proj_T("Wkt", "bkt", xtt, dst_dram=bk_in[0])
                ag(bk_in[0], gath_k[0])
                proj_T("Wqi", "bqi", xti, dst_sb=qt)
                proj_V("Wvt", "bvt", xtt, bv_in[0])
                ag(bv_in[0], gath_v[0])
                proj_T("Wki", "bki", xti, dst_dram=bk_in[1])
                ag(bk_in[1], gath_k[1])
                proj_V("Wvi", "bvi", xti, bv_in[1])
                ag(bv_in[1], gath_v[1])
                proj_T("Wqt", "bqt", xtt, dst_dram=qTt_d)

            # ============== stage 2: attention per branch ==============
            with (
                tc.tile_pool(name="attn", bufs=1) as attn,
                tc.tile_pool(name="ps2", bufs=2, space="PSUM") as psp,
            ):
                def attention(b, gk, gv):
                    # b=0: image queries -> attended_tabular -> fused chunks 8..15
                    foc = 8 if b == 0 else 0
                    attacc = attn.tile([128, NCH, SH], f32, tag="acc")
                    l_ps = [
                        psp.tile(
                            [1, 512], f32, tag=f"l{qs}", name=f"l{qs}_{b}", bufs=1
                        )
                        for qs in range(2)
                    ]
                    for kh in range(2):
                        AT = attn.tile([128, 32, SH], bf16, tag="AT")
                        # ---- S^T phase: A^T[k, q] = exp(K @ Q^T - M)
                        for kb2 in range(16):
                            csrc = kh * 4 + kb2 // 4
                            j0 = (kb2 % 4) * 256
                            kt = attn.tile([128, NCH, 256], bf16, tag="kt", bufs=3)
                            nc.sync.dma_start(
                                out=kt[:],
                                in_=gk[csrc * SH : (csrc + 1) * SH, j0 : j0 + 256]
                                .rearrange("(dc p) k -> p dc k", p=128),
                            )
                            for ki in range(2):
                                kbl = kb2 * 2 + ki
                                psA = psp.tile([128, 512], f32, tag="mmA")
                                psB = psp.tile([128, 512], f32, tag="mmB")
                                for c in range(NCH):
                                    lhsT = kt[:, c, ki * 128 : (ki + 1) * 128]
                                    nc.tensor.matmul(
                                        psA[:], lhsT, qt[:, c, 0:512],
                                        start=(c == 0), stop=(c == NCH - 1),
                                    )
                                    nc.tensor.matmul(
                                        psB[:], lhsT, qt[:, c, 512:1024],
                                        start=(c == 0), stop=(c == NCH - 1),
                                    )
                                for qs, ps in enumerate((psA, psB)):
                                    nc.scalar.activation(
                                        AT[:, kbl, qs * 512 : (qs + 1) * 512],
                                        ps[:], EXP,
                                        bias=negm[:, 0:1], scale=1.0,
                                    )
                        # ---- l burst: l[q] += sum_k A^T[k, q]
                        for kbl in range(32):
                            for qs in range(2):
                                nc.tensor.matmul(
                                    l_ps[qs][:],
                                    ones_col[:, 0:1],
                                    AT[:, kbl, qs * 512 : (qs + 1) * 512],
                                    start=(kh == 0 and kbl == 0),
                                    stop=(kh == 1 and kbl == 31),
                                    skip_group_check=True,
                                )
                        # ---- AV phase: attended^T[d, q] += V^T @ A^T
                        for dblk in range(NCH):
                            vp = attn.tile([128, 32, 128], bf16, tag="vp", bufs=2)
                            nc.scalar.dma_start(
                                out=vp[:],
                                in_=gv[
                                    kh * 4096 : (kh + 1) * 4096,
                                    dblk * 128 : (dblk + 1) * 128,
                                ].rearrange("(kb p) d -> p kb d", p=128),
                            )
                            avA = psp.tile([128, 512], f32, tag="mmA")
                            avB = psp.tile([128, 512], f32, tag="mmB")
                            for kbl in range(32):
                                lhsT = vp[:, kbl, :]
                                nc.tensor.matmul(
                                    avA[:], lhsT, AT[:, kbl, 0:512],
                                    start=(kbl == 0), stop=(kbl == 31),
                                )
                                nc.tensor.matmul(
                                    avB[:], lhsT, AT[:, kbl, 512:1024],
                                    start=(kbl == 0), stop=(kbl == 31),
                                )
                            for qs, av in enumerate((avA, avB)):
                                sl = slice(qs * 512, (qs + 1) * 512)
                                if kh == 0:
                                    nc.vector.tensor_copy(
                                        attacc[:, dblk, sl], av[:]
                                    )
                                else:
                                    nc.vector.scalar_tensor_tensor(
                                        fusedT[:, foc + dblk, sl],
                                        av[:], 1.0, attacc[:, dblk, sl],
                                        OP.bypass, OP.add,
                                    )
                    # ---- l -> linv as [q%128, q//128] column layout
                    lr = pp.tile([1, SH], f32, tag="lrow")
                    for qs in range(2):
                        nc.vector.tensor_copy(
                            lr[0:1, qs * 512 : (qs + 1) * 512], l_ps[qs][:]
                        )
                    nc.gpsimd.dma_start(out=l_d[b][:, :], in_=lr[:])
                    lcol = pp.tile([128, NCH], f32, tag="lcol")
                    nc.sync.dma_start(
                        out=lcol[:],
                        in_=l_d[b][0, :].rearrange("(c p) -> p c", p=128),
                    )
                    nc.vector.reciprocal(linv[b][:], lcol[:])

                attention(0, gath_k[0], gath_v[0])
                # branch 1 reloads q^T into the same slot (WAR handled by Tile)
                nc.scalar.dma_start(
                    out=qt[:],
                    in_=qTt_d[:, :].rearrange("(c p) x -> p c x", p=128),
                )
                attention(1, gath_k[1], gath_v[1])

            # ============== stage 3: output projection ==============
            with (
                tc.tile_pool(name="outp", bufs=1) as outp,
                tc.tile_pool(name="pso", bufs=2, space="PSUM") as pso,
            ):
                for oh in range(2):
                    wo = outp.tile([128, 16, D], bf16, tag="wo")
                    nc.sync.dma_start(
                        out=wo[:],
                        in_=Wo16[:, oh * D : (oh + 1) * D].rearrange(
                            "(c p) o -> p c o", p=128
                        ),
                    )
                    for qb in range(NCH):
                        psq = [
                            [
                                pso.tile(
                                    [128, 512], f32, tag=f"po{h}{obl}",
                                    name=f"po{h}{obl}",
                                )
                                for obl in range(2)
                            ]
                            for h in range(2)
                        ]
                        for c in range(16):
                            lhsT = fusedT[:, c, qb * 128 : (qb + 1) * 128]
                            h = 0 if c < 8 else 1
                            st = c % 8 == 0
                            sp = c % 8 == 7
                            for obl in range(2):
                                nc.tensor.matmul(
                                    psq[h][obl][:], lhsT,
                                    wo[:, c, obl * 512 : (obl + 1) * 512],
                                    start=st, stop=sp,
                                )
                        for obl in range(2):
                            o0 = oh * D + obl * 512
                            # chunks 0..7 = attended_image = branch 1;
                            # chunks 8..15 = attended_tabular = branch 0
                            t1 = outp.tile([128, 512], f32, tag="t1")
                            nc.vector.scalar_tensor_tensor(
                                t1[:], psq[1][obl][:], linv[0][:, qb : qb + 1],
                                bob_sb[:, o0 : o0 + 512], OP.mult, OP.add,
                            )
                            ost = outp.tile([128, 512], f32, tag="ost", bufs=4)
                            nc.vector.scalar_tensor_tensor(
                                ost[:], psq[0][obl][:], linv[1][:, qb : qb + 1],
                                t1[:], OP.mult, OP.add,
                            )
                            nc.gpsimd.dma_start(
                                out=out[qb * 128 : (qb + 1) * 128, o0 : o0 + 512],
                                in_=ost[:],
                            )

    nc.compile()
    return nc


_CACHE: dict = {}


def kernel(
    image_features, tabular_features,
    Wqi, bqi, Wkt, bkt, Wvt, bvt,
    Wqt, bqt, Wki, bki, Wvi, bvi,
    Wo, bo,
) -> np.ndarray:
    if "nc" not in _CACHE:
        _CACHE["nc"] = build_nc()
    nc = _CACHE["nc"]

    img = np.asarray(image_features, np.float32)
    tab = np.asarray(tabular_features, np.float32)

    def bcol(b):
        return np.ascontiguousarray(
            np.asarray(b, np.float32).reshape(NCH, 128).T
        )

    def brow(b):
        return np.ascontiguousarray(
            np.broadcast_to(
                np.asarray(b).astype(ml_dtypes.bfloat16).reshape(1, D), (128, D)
            )
        )

    shared = {
        "Wqi": np.asarray(Wqi, np.float32), "Wkt": np.asarray(Wkt, np.float32),
        "Wvt": np.asarray(Wvt, np.float32), "Wqt": np.asarray(Wqt, np.float32),
        "Wki": np.asarray(Wki, np.float32), "Wvi": np.asarray(Wvi, np.float32),
        "Wo16": np.asarray(Wo).astype(ml_dtypes.bfloat16),
        "bc_bqi": bcol(bqi), "bc_bkt": bcol(bkt),
        "bc_bqt": bcol(bqt), "bc_bki": bcol(bki),
        "br_bvt": brow(bvt), "br_bvi": brow(bvi),
        "bob": np.ascontiguousarray(
            np.broadcast_to(
                np.asarray(bo).astype(ml_dtypes.bfloat16).reshape(1, 2 * D),
                (128, 2 * D),
            )
        ),
    }
    in_maps = []
    for c in range(NCORES):
        m = dict(shared)
        m["xTi"] = np.ascontiguousarray(img[c * SH : (c + 1) * SH, :].T)
        m["xTt"] = np.ascontiguousarray(tab[c * SH : (c + 1) * SH, :].T)
        in_maps.append(m)

    trace = bool(int(os.environ.get("KERNEL_TRACE", "0")))
    res = run_bass_kernel_spmd(
        nc, in_maps, core_ids=list(range(NCORES)), trace=trace
    )
    _CACHE["last_result"] = res
    return np.concatenate([res.results[c]["out"] for c in range(NCORES)], axis=0)


# revision 21
# speedup vs baseline: 1.4213x; 1.0530x over previous
"""Co-attention fusion kernel for 8 TRN2 NeuronCores.

Strategy (row-parallel flash attention per the sharding hint):
- Shard rows (N=8192) of image/tabular features across 8 cores (1024 each).
- Each core computes its local K^T / V / Q projection shards in f32r,
  casts them to bf16, AllGathers K^T and V (bf16), then computes its 1024
  query rows against the full gathered keys/values plus the output
  projection for its row shard.

Layout trick: S is computed TRANSPOSED (S^T = K @ Q^T, keys on partitions)
so the A@V phase needs no PE transposes at all: attended^T = V^T @ A^T uses
V tiles as the stationary operand directly in natural [key, d] layout, and
attended^T chunks feed the output projection as stationary operands too.
All attention-phase matmuls are bf16 (FWL weight loads overlap streaming).

Numerics: Q/K in bf16 give logit error ~0.14; with the peaked softmax here
(logit std ~13, top-2 gaps ~5) the CPU-simulated end-to-end rel err is
0.0095 vs the 2e-2 gate. Softmax uses a fixed shift M=96 (row maxima are
44..87, so exp(s-96) never overflows and stays in bf16/fp32 range); the
1/l normalization is applied per-query at the *output projection* drain
(per-partition scalar), per branch, fused with the bias add on the DVE.
"""

import os
import numpy as np
import ml_dtypes

import concourse.bacc as bacc
import concourse.mybir as mybir
import concourse.tile as tile
from concourse.bass_utils import run_bass_kernel_spmd

N = 8192
D = 1024
NCORES = 8
SH = N // NCORES  # 1024 rows per core
NCH = D // 128  # 8 contraction chunks
M_SHIFT = 96.0  # softmax shift (see module docstring)

f32 = mybir.dt.float32
f32r = mybir.dt.float32r
bf16 = mybir.dt.bfloat16

OP = mybir.AluOpType
EXP = mybir.ActivationFunctionType.Exp


def build_nc():
    nc = bacc.Bacc(trn_type="TRN2", num_devices=NCORES)

    # ---- parameters ----
    xTi = nc.declare_dram_parameter("xTi", [D, SH], f32, isOutput=False)
    xTt = nc.declare_dram_parameter("xTt", [D, SH], f32, isOutput=False)
    Ws = {
        name: nc.declare_dram_parameter(name, [D, D], f32, isOutput=False)
        for name in ["Wqi", "Wkt", "Wvt", "Wqt", "Wki", "Wvi"]
    }
    Wo16 = nc.declare_dram_parameter("Wo16", [2 * D, 2 * D], bf16, isOutput=False)
    # q/k biases in column layout [dout%128, dout//128]
    Bcol = {
        name: nc.declare_dram_parameter("bc_" + name, [128, NCH], f32, isOutput=False)
        for name in ["bqi", "bkt", "bqt", "bki"]
    }
    # v biases broadcast across partitions
    Brow = {
        name: nc.declare_dram_parameter("br_" + name, [128, D], bf16, isOutput=False)
        for name in ["bvt", "bvi"]
    }
    bob = nc.declare_dram_parameter("bob", [128, 2 * D], bf16, isOutput=False)
    out = nc.declare_dram_parameter("out", [SH, 2 * D], f32, isOutput=True)

    # ---- internal DRAM ----
    bk_in = [nc.dram_tensor(f"bk_in{i}", [D, SH], bf16) for i in range(2)]
    bv_in = [nc.dram_tensor(f"bv_in{i}", [SH, D], bf16) for i in range(2)]
    qTt_d = nc.dram_tensor("qTt_d", [D, SH], bf16)
    l_d = [nc.dram_tensor(f"l_d{i}", [1, SH], f32) for i in range(2)]
    gath_k = [
        nc.dram_tensor(f"gath_k{i}", [N, SH], bf16, addr_space="Shared")
        for i in range(2)
    ]
    gath_v = [
        nc.dram_tensor(f"gath_v{i}", [N, D], bf16, addr_space="Shared")
        for i in range(2)
    ]

    rg = [list(range(NCORES))]

    def ag(src_t, dst_t):
        nc.gpsimd.collective_compute(
            "AllGather",
            OP.bypass,
            replica_groups=rg,
            ins=[src_t.ap().opt()],
            outs=[dst_t.ap().opt()],
        )

    with tile.TileContext(nc) as tc:
        with tc.tile_pool(name="pp", bufs=1) as pp:
            # ---- long-lived tiles ----
            negm = pp.tile([128, 1], f32, tag="negm")
            nc.vector.memset(negm[:], -M_SHIFT)
            ones_f32 = pp.tile([128, 1], f32, tag="ones")
            nc.vector.memset(ones_f32[:], 1.0)
            qt = pp.tile([128, NCH, SH], bf16, tag="qt")
            fusedT = pp.tile([128, 16, SH], bf16, tag="fusedT")
            bob_sb = pp.tile([128, 2 * D], bf16, tag="bob")
            nc.scalar.dma_start(out=bob_sb[:], in_=bob[:, :])
            linv = [
                pp.tile([128, NCH], f32, tag=f"linv{b}", name=f"linv{b}")
                for b in range(2)
            ]

            # ============== stage 1: projections + AllGather ==============
            with (
                tc.tile_pool(name="s1", bufs=1) as s1,
                tc.tile_pool(name="ps1", bufs=2, space="PSUM") as psp,
            ):
                xti = s1.tile([128, NCH, SH], f32r, tag="xti")
                xtt = s1.tile([128, NCH, SH], f32r, tag="xtt")
                nc.sync.dma_start(
                    out=xti[:],
                    in_=xTi[:, :].rearrange("(c p) x -> p c x", p=128).bitcast(f32r),
                )
                nc.sync.dma_start(
                    out=xtt[:],
                    in_=xTt[:, :].rearrange("(c p) x -> p c x", p=128).bitcast(f32r),
                )
                bcol = {}
                for bn in Bcol:
                    bcol[bn] = s1.tile([128, NCH], f32, tag="bc" + bn, name="bc_" + bn)
                    nc.sync.dma_start(out=bcol[bn][:], in_=Bcol[bn][:, :])
                brow = {}
                for bn in Brow:
                    brow[bn] = s1.tile([128, D], bf16, tag="br" + bn, name="br_" + bn)
                    nc.sync.dma_start(out=brow[bn][:], in_=Brow[bn][:, :])

                def load_w(wname):
                    """W in four 256-col quarters as separate f32r tiles."""
                    wqs = []
                    for wq in range(4):
                        w = s1.tile(
                            [128, NCH, 256], f32r, tag="w", bufs=6,
                            name=f"w_{wname}{wq}",
                        )
                        nc.sync.dma_start(
                            out=w[:],
                            in_=Ws[wname][:, wq * 256 : (wq + 1) * 256]
                            .rearrange("(c p) x -> p c x", p=128)
                            .bitcast(f32r),
                        )
                        wqs.append(w)
                    return wqs

                def proj_T(wname, bname, xt, dst_dram=None, dst_sb=None):
                    """q^T/k^T projection: out[d_out, rows] bf16 blocks."""
                    wqs = load_w(wname)
                    for od in range(NCH):
                        w = wqs[od // 2]
                        odl = od % 2
                        psA = psp.tile([128, 512], f32, tag="mmA")
                        psB = psp.tile([128, 512], f32, tag="mmB")
                        for c in range(NCH):
                            lhsT = w[:, c, odl * 128 : (odl + 1) * 128]
                            nc.tensor.matmul(
                                psA[:], lhsT, xt[:, c, 0:512],
                                start=(c == 0), stop=(c == NCH - 1),
                            )
                            nc.tensor.matmul(
                                psB[:], lhsT, xt[:, c, 512:1024],
                                start=(c == 0), stop=(c == NCH - 1),
                            )
                        for rt, ps in enumerate((psA, psB)):
                            if dst_sb is not None:
                                nc.vector.tensor_scalar_add(
                                    dst_sb[:, od, rt * 512 : (rt + 1) * 512],
                                    ps[:], bcol[bname][:, od : od + 1],
                                )
                            else:
                                stg = s1.tile([128, 512], bf16, tag="stgT", bufs=4)
                                nc.vector.tensor_scalar_add(
                                    stg[:], ps[:], bcol[bname][:, od : od + 1]
                                )
                                nc.gpsimd.dma_start(
                                    out=dst_dram[
                                        od * 128 : (od + 1) * 128,
                                        rt * 512 : (rt + 1) * 512,
                                    ],
                                    in_=stg[:],
                                )

                def proj_V(wname, bname, xt, dst_bv):
                    """v projection, natural [rows, d_out] bf16 blocks.

                    The four 256-wide W quarters accumulate into two PSUM
                    banks (two 256-col column-groups per bank), so only the
                    very first matmul of each bank carries start=True.
                    """
                    wqs = load_w(wname)
                    for rt in range(NCH):
                        psA = psp.tile([128, 512], f32, tag="mmA")
                        psB = psp.tile([128, 512], f32, tag="mmB")
                        for c in range(NCH):
                            lhsT = xt[:, c, rt * 128 : (rt + 1) * 128]
                            for wq in range(4):
                                ps = psA if wq < 2 else psB
                                cs = slice((wq % 2) * 256, (wq % 2) * 256 + 256)
                                nc.tensor.matmul(
                                    ps[:, cs], lhsT, wqs[wq][:, c, :],
                                    start=(c == 0 and wq % 2 == 0),
                                    stop=(c == NCH - 1 and wq % 2 == 1),
                                    skip_group_check=True,
                                )
                        stg = s1.tile([128, D], bf16, tag="stgV", bufs=4)
                        for oh, ps in enumerate((psA, psB)):
                            nc.vector.scalar_tensor_tensor(
                                stg[:, oh * 512 : (oh + 1) * 512],
                                ps[:], 1.0,
                                brow[bname][:, oh * 512 : (oh + 1) * 512],
                                OP.bypass, OP.add,
                            )
                        nc.gpsimd.dma_start(
                            out=dst_bv[rt * 128 : (rt + 1) * 128, :], in_=stg[:]
                        )

                # Branch-0 K and V first; each AllGather queues right after its
                # projection so the collectives drain while the PE projects.
                proj_T("Wkt", "bkt", xtt, dst_dram=bk_in[0])
                ag(bk_in[0], gath_k[0])
                proj_V("Wvt", "bvt", xtt, bv_in[0])
                ag(bv_in[0], gath_v[0])
                proj_T("Wqi", "bqi", xti, dst_sb=qt)
                proj_T("Wki", "bki", xti, dst_dram=bk_in[1])
                ag(bk_in[1], gath_k[1])
                proj_V("Wvi", "bvi", xti, bv_in[1])
                ag(bv_in[1], gath_v[1])
                proj_T("Wqt", "bqt", xtt, dst_dram=qTt_d)

            # ============== stage 2: attention per branch ==============
            with (
                tc.tile_pool(name="attn", bufs=1) as attn,
                tc.tile_pool(name="ps2", bufs=2, space="PSUM") as psp,
            ):
                def attention(b, gk, gv):
                    # b=0: image queries -> attended_tabular -> fused chunks 8..15
                    foc = 8 if b == 0 else 0
                    attacc = attn.tile([128, NCH, SH], f32, tag="acc")
                    l_acc = attn.tile([128, SH], f32, tag="lacc")
                    l_ps = [
                        psp.tile(
                            [1, 512], f32, tag=f"l{qs}", name=f"l{qs}_{b}", bufs=1
                        )
                        for qs in range(2)
                    ]
                    for kh in range(2):
                        AT = attn.tile([128, 32, SH], bf16, tag="AT")
                        # ---- S^T phase: A^T[k, q] = exp(K @ Q^T - M)
                        for kb2 in range(16):
                            csrc = kh * 4 + kb2 // 4
                            j0 = (kb2 % 4) * 256
                            kt = attn.tile([128, NCH, 256], bf16, tag="kt", bufs=3)
                            nc.sync.dma_start(
                                out=kt[:],
                                in_=gk[csrc * SH : (csrc + 1) * SH, j0 : j0 + 256]
                                .rearrange("(dc p) k -> p dc k", p=128),
                            )
                            for ki in range(2):
                                kbl = kb2 * 2 + ki
                                psA = psp.tile([128, 512], f32, tag="mmA")
                                psB = psp.tile([128, 512], f32, tag="mmB")
                                for c in range(NCH):
                                    lhsT = kt[:, c, ki * 128 : (ki + 1) * 128]
                                    nc.tensor.matmul(
                                        psA[:], lhsT, qt[:, c, 0:512],
                                        start=(c == 0), stop=(c == NCH - 1),
                                    )
                                    nc.tensor.matmul(
                                        psB[:], lhsT, qt[:, c, 512:1024],
                                        start=(c == 0), stop=(c == NCH - 1),
                                    )
                                for qs, ps in enumerate((psA, psB)):
                                    sl = slice(qs * 512, (qs + 1) * 512)
                                    nc.scalar.activation(
                                        AT[:, kbl, sl], ps[:], EXP,
                                        bias=negm[:, 0:1], scale=1.0,
                                    )
                                    # per-partition partial row sums on DVE
                                    # (keeps the l reduction off the PE)
                                    if kh == 0 and kbl == 0:
                                        nc.vector.tensor_copy(
                                            l_acc[:, sl], AT[:, kbl, sl]
                                        )
                                    else:
                                        nc.vector.scalar_tensor_tensor(
                                            l_acc[:, sl], AT[:, kbl, sl], 1.0,
                                            l_acc[:, sl], OP.bypass, OP.add,
                                        )
                        # ---- AV phase: attended^T[d, q] += V^T @ A^T
                        for dblk in range(NCH):
                            vp = attn.tile([128, 32, 128], bf16, tag="vp", bufs=2)
                            nc.scalar.dma_start(
                                out=vp[:],
                                in_=gv[
                                    kh * 4096 : (kh + 1) * 4096,
                                    dblk * 128 : (dblk + 1) * 128,
                                ].rearrange("(kb p) d -> p kb d", p=128),
                            )
                            avA = psp.tile([128, 512], f32, tag="mmA")
                            avB = psp.tile([128, 512], f32, tag="mmB")
                            for kbl in range(32):
                                lhsT = vp[:, kbl, :]
                                nc.tensor.matmul(
                                    avA[:], lhsT, AT[:, kbl, 0:512],
                                    start=(kbl == 0), stop=(kbl == 31),
                                )
                                nc.tensor.matmul(
                                    avB[:], lhsT, AT[:, kbl, 512:1024],
                                    start=(kbl == 0), stop=(kbl == 31),
                                )
                            for qs, av in enumerate((avA, avB)):
                                sl = slice(qs * 512, (qs + 1) * 512)
                                if kh == 0:
                                    nc.vector.tensor_copy(
                                        attacc[:, dblk, sl], av[:]
                                    )
                                else:
                                    nc.vector.scalar_tensor_tensor(
                                        fusedT[:, foc + dblk, sl],
                                        av[:], 1.0, attacc[:, dblk, sl],
                                        OP.bypass, OP.add,
                                    )
                    # ---- l -> linv as [q%128, q//128] column layout
                    for qs in range(2):
                        nc.tensor.matmul(
                            l_ps[qs][:], ones_f32[:, 0:1],
                            l_acc[:, qs * 512 : (qs + 1) * 512],
                            start=True, stop=True,
                        )
                    lr = pp.tile([1, SH], f32, tag="lrow")
                    for qs in range(2):
                        nc.vector.tensor_copy(
                            lr[0:1, qs * 512 : (qs + 1) * 512], l_ps[qs][:]
                        )
                    nc.gpsimd.dma_start(out=l_d[b][:, :], in_=lr[:])
                    lcol = pp.tile([128, NCH], f32, tag="lcol")
                    nc.sync.dma_start(
                        out=lcol[:],
                        in_=l_d[b][0, :].rearrange("(c p) -> p c", p=128),
                    )
                    nc.vector.reciprocal(linv[b][:], lcol[:])

                attention(0, gath_k[0], gath_v[0])
                # branch 1 reloads q^T into the same slot (WAR handled by Tile)
                nc.scalar.dma_start(
                    out=qt[:],
                    in_=qTt_d[:, :].rearrange("(c p) x -> p c x", p=128),
                )
                attention(1, gath_k[1], gath_v[1])

            # ============== stage 3: output projection ==============
            with (
                tc.tile_pool(name="outp", bufs=1) as outp,
                tc.tile_pool(name="pso", bufs=2, space="PSUM") as pso,
            ):
                for oq in range(4):
                    wo = outp.tile([128, 16, 512], bf16, tag="wo", bufs=3)
                    nc.sync.dma_start(
                        out=wo[:],
                        in_=Wo16[:, oq * 512 : (oq + 1) * 512].rearrange(
                            "(c p) o -> p c o", p=128
                        ),
                    )
                    for qb in range(NCH):
                        psA = pso.tile([128, 512], f32, tag="poA")
                        psB = pso.tile([128, 512], f32, tag="poB")
                        for c in range(16):
                            lhsT = fusedT[:, c, qb * 128 : (qb + 1) * 128]
                            ps = psA if c < 8 else psB
                            nc.tensor.matmul(
                                ps[:], lhsT, wo[:, c, :],
                                start=(c % 8 == 0), stop=(c % 8 == 7),
                            )
                        o0 = oq * 512
                        # chunks 0..7 = attended_image = branch 1;
                        # chunks 8..15 = attended_tabular = branch 0
                        t1 = outp.tile([128, 512], f32, tag="t1")
                        nc.vector.scalar_tensor_tensor(
                            t1[:], psB[:], linv[0][:, qb : qb + 1],
                            bob_sb[:, o0 : o0 + 512], OP.mult, OP.add,
                        )
                        ost = outp.tile([128, 512], f32, tag="ost", bufs=4)
                        nc.vector.scalar_tensor_tensor(
                            ost[:], psA[:], linv[1][:, qb : qb + 1],
                            t1[:], OP.mult, OP.add,
                        )
                        nc.gpsimd.dma_start(
                            out=out[qb * 128 : (qb + 1) * 128, o0 : o0 + 512],
                            in_=ost[:],
                        )

    nc.compile()
    return nc


_CACHE: dict = {}


def kernel(
    image_features, tabular_features,
    Wqi, bqi, Wkt, bkt, Wvt, bvt,
    Wqt, bqt, Wki, bki, Wvi, bvi,
    Wo, bo,
) -> np.ndarray:
    if "nc" not in _CACHE:
        _CACHE["nc"] = build_nc()
    nc = _CACHE["nc"]

    img = np.asarray(image_features, np.float32)
    tab = np.asarray(tabular_features, np.float32)

    def bcol(b):
        return np.ascontiguousarray(
            np.asarray(b, np.float32).reshape(NCH, 128).T
        )

    def brow(b):
        return np.ascontiguousarray(
            np.broadcast_to(
                np.asarray(b).astype(ml_dtypes.bfloat16).reshape(1, D), (128, D)
            )
        )

    shared = {
        "Wqi": np.asarray(Wqi, np.float32), "Wkt": np.asarray(Wkt, np.float32),
        "Wvt": np.asarray(Wvt, np.float32), "Wqt": np.asarray(Wqt, np.float32),
        "Wki": np.asarray(Wki, np.float32), "Wvi": np.asarray(Wvi, np.float32),
        "Wo16": np.asarray(Wo).astype(ml_dtypes.bfloat16),
        "bc_bqi": bcol(bqi), "bc_bkt": bcol(bkt),
        "bc_bqt": bcol(bqt), "bc_bki": bcol(bki),
        "br_bvt": brow(bvt), "br_bvi": brow(bvi),
        "bob": np.ascontiguousarray(
            np.broadcast_to(
                np.asarray(bo).astype(ml_dtypes.bfloat16).reshape(1, 2 * D),
                (128, 2 * D),
            )
        ),
    }
    in_maps = []
    for c in range(NCORES):
        m = dict(shared)
        m["xTi"] = np.ascontiguousarray(img[c * SH : (c + 1) * SH, :].T)
        m["xTt"] = np.ascontiguousarray(tab[c * SH : (c + 1) * SH, :].T)
        in_maps.append(m)

    trace = bool(int(os.environ.get("KERNEL_TRACE", "0")))
    res = run_bass_kernel_spmd(
        nc, in_maps, core_ids=list(range(NCORES)), trace=trace
    )
    _CACHE["last_result"] = res
    return np.concatenate([res.results[c]["out"] for c in range(NCORES)], axis=0)


# revision 27
# speedup vs baseline: 1.4245x; 1.0022x over previous
"""Co-attention fusion kernel for 8 TRN2 NeuronCores.

Strategy (row-parallel flash attention per the sharding hint):
- Shard rows (N=8192) of image/tabular features across 8 cores (1024 each).
- Each core computes its local K^T / V / Q projection shards in f32r,
  casts them to bf16, AllGathers K^T and V (bf16), then computes its 1024
  query rows against the full gathered keys/values plus the output
  projection for its row shard.

Layout trick: S is computed TRANSPOSED (S^T = K @ Q^T, keys on partitions)
so the A@V phase needs no PE transposes at all: attended^T = V^T @ A^T uses
V tiles as the stationary operand directly in natural [key, d] layout, and
attended^T chunks feed the output projection as stationary operands too.
All attention-phase matmuls are bf16 (FWL weight loads overlap streaming).

Numerics: Q/K in bf16 give logit error ~0.14; with the peaked softmax here
(logit std ~13, top-2 gaps ~5) the CPU-simulated end-to-end rel err is
0.0095 vs the 2e-2 gate. Softmax uses a fixed shift M=96 (row maxima are
44..87, so exp(s-96) never overflows and stays in bf16/fp32 range); the
1/l normalization is applied per-query at the *output projection* drain
(per-partition scalar), per branch, fused with the bias add on the DVE.
"""

import os
import numpy as np
import ml_dtypes

import concourse.bacc as bacc
import concourse.mybir as mybir
import concourse.tile as tile
from concourse.bass_utils import run_bass_kernel_spmd

N = 8192
D = 1024
NCORES = 8
SH = N // NCORES  # 1024 rows per core
NCH = D // 128  # 8 contraction chunks
M_SHIFT = 96.0  # softmax shift (see module docstring)

f32 = mybir.dt.float32
f32r = mybir.dt.float32r
bf16 = mybir.dt.bfloat16

OP = mybir.AluOpType
EXP = mybir.ActivationFunctionType.Exp


def build_nc():
    nc = bacc.Bacc(trn_type="TRN2", num_devices=NCORES)

    # ---- parameters ----
    xTi = nc.declare_dram_parameter("xTi", [D, SH], f32, isOutput=False)
    xTt = nc.declare_dram_parameter("xTt", [D, SH], f32, isOutput=False)
    Ws = {
        name: nc.declare_dram_parameter(name, [D, D], f32, isOutput=False)
        for name in ["Wqi", "Wkt", "Wvt", "Wqt", "Wki", "Wvi"]
    }
    Wo16 = nc.declare_dram_parameter("Wo16", [2 * D, 2 * D], bf16, isOutput=False)
    # q/k biases in column layout [dout%128, dout//128], packed bqi|bkt|bqt|bki
    Bcol = nc.declare_dram_parameter("bcol", [128, 4 * NCH], f32, isOutput=False)
    # v biases broadcast across partitions, packed bvt|bvi
    Brow = nc.declare_dram_parameter("brow", [128, 2 * D], bf16, isOutput=False)
    bob = nc.declare_dram_parameter("bob", [128, 2 * D], bf16, isOutput=False)
    out = nc.declare_dram_parameter("out", [SH, 2 * D], f32, isOutput=True)

    # ---- internal DRAM ----
    bk_in = [nc.dram_tensor(f"bk_in{i}", [D, SH], bf16) for i in range(2)]
    bv_in = [nc.dram_tensor(f"bv_in{i}", [SH, D], bf16) for i in range(2)]
    qTt_d = nc.dram_tensor("qTt_d", [D, SH], bf16)
    l_d = [nc.dram_tensor(f"l_d{i}", [1, SH], f32) for i in range(2)]
    gath_k = [
        nc.dram_tensor(f"gath_k{i}", [N, SH], bf16, addr_space="Shared")
        for i in range(2)
    ]
    gath_v = [
        nc.dram_tensor(f"gath_v{i}", [N, D], bf16, addr_space="Shared")
        for i in range(2)
    ]

    rg = [list(range(NCORES))]

    def ag(src_t, dst_t):
        nc.gpsimd.collective_compute(
            "AllGather",
            OP.bypass,
            replica_groups=rg,
            ins=[src_t.ap().opt()],
            outs=[dst_t.ap().opt()],
        )

    with tile.TileContext(nc) as tc:
        with tc.tile_pool(name="pp", bufs=1) as pp:
            # ---- long-lived tiles ----
            negm = pp.tile([128, 1], f32, tag="negm")
            nc.vector.memset(negm[:], -M_SHIFT)
            ones_f32 = pp.tile([128, 1], f32, tag="ones")
            nc.vector.memset(ones_f32[:], 1.0)
            qt = pp.tile([128, NCH, SH], bf16, tag="qt")
            fusedT = pp.tile([128, 16, SH], bf16, tag="fusedT")
            bob_sb = pp.tile([128, 2 * D], bf16, tag="bob")
            nc.scalar.dma_start(out=bob_sb[:], in_=bob[:, :])
            linv = [
                pp.tile([128, NCH], f32, tag=f"linv{b}", name=f"linv{b}")
                for b in range(2)
            ]

            # ============== stage 1: projections + AllGather ==============
            with (
                tc.tile_pool(name="s1", bufs=1) as s1,
                tc.tile_pool(name="ps1", bufs=2, space="PSUM") as psp,
            ):
                xti = s1.tile([128, NCH, SH], f32r, tag="xti")
                xtt = s1.tile([128, NCH, SH], f32r, tag="xtt")
                nc.scalar.dma_start(
                    out=xti[:],
                    in_=xTi[:, :].rearrange("(c p) x -> p c x", p=128).bitcast(f32r),
                )
                nc.scalar.dma_start(
                    out=xtt[:],
                    in_=xTt[:, :].rearrange("(c p) x -> p c x", p=128).bitcast(f32r),
                )
                bcall = s1.tile([128, 4 * NCH], f32, tag="bcall")
                nc.sync.dma_start(out=bcall[:], in_=Bcol[:, :])
                bcoff = {
                    bn: i * NCH for i, bn in enumerate(["bqi", "bkt", "bqt", "bki"])
                }
                brall = s1.tile([128, 2 * D], bf16, tag="brall")
                nc.sync.dma_start(out=brall[:], in_=Brow[:, :])
                broff = {"bvt": 0, "bvi": D}

                def load_w(wname):
                    """W in two 512-col halves as separate f32r tiles."""
                    whs = []
                    for wh in range(2):
                        w = s1.tile(
                            [128, NCH, 512], f32r, tag="w", bufs=3,
                            name=f"w_{wname}{wh}",
                        )
                        nc.scalar.dma_start(
                            out=w[:],
                            in_=Ws[wname][:, wh * 512 : (wh + 1) * 512]
                            .rearrange("(c p) x -> p c x", p=128)
                            .bitcast(f32r),
                        )
                        whs.append(w)
                    return whs

                def proj_T(wname, bname, xt, dst_dram=None, dst_sb=None):
                    """q^T/k^T projection: out[d_out, rows] bf16 blocks."""
                    wqs = load_w(wname)
                    for od in range(NCH):
                        w = wqs[od // 4]
                        odl = od % 4
                        psA = psp.tile([128, 512], f32, tag="mmA")
                        psB = psp.tile([128, 512], f32, tag="mmB")
                        for c in range(NCH):
                            lhsT = w[:, c, odl * 128 : (odl + 1) * 128]
                            nc.tensor.matmul(
                                psA[:], lhsT, xt[:, c, 0:512],
                                start=(c == 0), stop=(c == NCH - 1),
                            )
                            nc.tensor.matmul(
                                psB[:], lhsT, xt[:, c, 512:1024],
                                start=(c == 0), stop=(c == NCH - 1),
                            )
                        bsl = slice(bcoff[bname] + od, bcoff[bname] + od + 1)
                        for rt, ps in enumerate((psA, psB)):
                            if dst_sb is not None:
                                nc.vector.tensor_scalar_add(
                                    dst_sb[:, od, rt * 512 : (rt + 1) * 512],
                                    ps[:], bcall[:, bsl],
                                )
                            else:
                                stg = s1.tile([128, 512], bf16, tag="stgT", bufs=4)
                                nc.vector.tensor_scalar_add(
                                    stg[:], ps[:], bcall[:, bsl]
                                )
                                nc.gpsimd.dma_start(
                                    out=dst_dram[
                                        od * 128 : (od + 1) * 128,
                                        rt * 512 : (rt + 1) * 512,
                                    ],
                                    in_=stg[:],
                                )

                def proj_V(wname, bname, xt, dst_bv):
                    """v projection, natural [rows, d_out] bf16 blocks."""
                    whs = load_w(wname)
                    for rt in range(NCH):
                        psA = psp.tile([128, 512], f32, tag="mmA")
                        psB = psp.tile([128, 512], f32, tag="mmB")
                        for c in range(NCH):
                            lhsT = xt[:, c, rt * 128 : (rt + 1) * 128]
                            nc.tensor.matmul(
                                psA[:], lhsT, whs[0][:, c, :],
                                start=(c == 0), stop=(c == NCH - 1),
                            )
                            nc.tensor.matmul(
                                psB[:], lhsT, whs[1][:, c, :],
                                start=(c == 0), stop=(c == NCH - 1),
                            )
                        stg = s1.tile([128, D], bf16, tag="stgV", bufs=4)
                        for oh, ps in enumerate((psA, psB)):
                            nc.vector.scalar_tensor_tensor(
                                stg[:, oh * 512 : (oh + 1) * 512],
                                ps[:], 1.0,
                                brall[:, broff[bname] + oh * 512 : broff[bname] + (oh + 1) * 512],
                                OP.bypass, OP.add,
                            )
                        nc.gpsimd.dma_start(
                            out=dst_bv[rt * 128 : (rt + 1) * 128, :], in_=stg[:]
                        )

                # Branch-0 K and V first; each AllGather queues right after its
                # projection so the collectives drain while the PE projects.
                proj_T("Wkt", "bkt", xtt, dst_dram=bk_in[0])
                ag(bk_in[0], gath_k[0])
                proj_V("Wvt", "bvt", xtt, bv_in[0])
                ag(bv_in[0], gath_v[0])
                proj_T("Wqi", "bqi", xti, dst_sb=qt)
                proj_T("Wki", "bki", xti, dst_dram=bk_in[1])
                ag(bk_in[1], gath_k[1])
                proj_V("Wvi", "bvi", xti, bv_in[1])
                ag(bv_in[1], gath_v[1])
                proj_T("Wqt", "bqt", xtt, dst_dram=qTt_d)

            # ============== stage 2: attention per branch ==============
            with (
                tc.tile_pool(name="attn", bufs=1) as attn,
                tc.tile_pool(name="ps2", bufs=2, space="PSUM") as psp,
            ):
                def attention(b, gk, gv):
                    # b=0: image queries -> attended_tabular -> fused chunks 8..15
                    foc = 8 if b == 0 else 0
                    attacc = attn.tile([128, NCH, SH], f32, tag="acc")
                    l_acc = attn.tile([128, SH], f32, tag="lacc")
                    l_ps = [
                        psp.tile(
                            [1, 512], f32, tag=f"l{qs}", name=f"l{qs}_{b}", bufs=1
                        )
                        for qs in range(2)
                    ]
                    for kh in range(2):
                        AT = attn.tile([128, 32, SH], bf16, tag="AT")
                        # ---- S^T phase: A^T[k, q] = exp(K @ Q^T - M)
                        for kb2 in range(16):
                            csrc = kh * 4 + kb2 // 4
                            j0 = (kb2 % 4) * 256
                            kt = attn.tile([128, NCH, 256], bf16, tag="kt", bufs=3)
                            nc.sync.dma_start(
                                out=kt[:],
                                in_=gk[csrc * SH : (csrc + 1) * SH, j0 : j0 + 256]
                                .rearrange("(dc p) k -> p dc k", p=128),
                            )
                            for ki in range(2):
                                kbl = kb2 * 2 + ki
                                psA = psp.tile([128, 512], f32, tag="mmA")
                                psB = psp.tile([128, 512], f32, tag="mmB")
                                for c in range(NCH):
                                    lhsT = kt[:, c, ki * 128 : (ki + 1) * 128]
                                    nc.tensor.matmul(
                                        psA[:], lhsT, qt[:, c, 0:512],
                                        start=(c == 0), stop=(c == NCH - 1),
                                    )
                                    nc.tensor.matmul(
                                        psB[:], lhsT, qt[:, c, 512:1024],
                                        start=(c == 0), stop=(c == NCH - 1),
                                    )
                                for qs, ps in enumerate((psA, psB)):
                                    sl = slice(qs * 512, (qs + 1) * 512)
                                    nc.scalar.activation(
                                        AT[:, kbl, sl], ps[:], EXP,
                                        bias=negm[:, 0:1], scale=1.0,
                                    )
                                    # per-partition partial row sums on DVE
                                    # (keeps the l reduction off the PE)
                                    if kh == 0 and kbl == 0:
                                        nc.vector.tensor_copy(
                                            l_acc[:, sl], AT[:, kbl, sl]
                                        )
                                    else:
                                        nc.vector.scalar_tensor_tensor(
                                            l_acc[:, sl], AT[:, kbl, sl], 1.0,
                                            l_acc[:, sl], OP.bypass, OP.add,
                                        )
                        # ---- AV phase: attended^T[d, q] += V^T @ A^T
                        for dblk in range(NCH):
                            vp = attn.tile([128, 32, 128], bf16, tag="vp", bufs=2)
                            nc.scalar.dma_start(
                                out=vp[:],
                                in_=gv[
                                    kh * 4096 : (kh + 1) * 4096,
                                    dblk * 128 : (dblk + 1) * 128,
                                ].rearrange("(kb p) d -> p kb d", p=128),
                            )
                            avA = psp.tile([128, 512], f32, tag="mmA")
                            avB = psp.tile([128, 512], f32, tag="mmB")
                            for kbl in range(32):
                                lhsT = vp[:, kbl, :]
                                nc.tensor.matmul(
                                    avA[:], lhsT, AT[:, kbl, 0:512],
                                    start=(kbl == 0), stop=(kbl == 31),
                                )
                                nc.tensor.matmul(
                                    avB[:], lhsT, AT[:, kbl, 512:1024],
                                    start=(kbl == 0), stop=(kbl == 31),
                                )
                            for qs, av in enumerate((avA, avB)):
                                sl = slice(qs * 512, (qs + 1) * 512)
                                if kh == 0:
                                    nc.vector.tensor_copy(
                                        attacc[:, dblk, sl], av[:]
                                    )
                                else:
                                    nc.vector.scalar_tensor_tensor(
                                        fusedT[:, foc + dblk, sl],
                                        av[:], 1.0, attacc[:, dblk, sl],
                                        OP.bypass, OP.add,
                                    )
                    # ---- l -> linv as [q%128, q//128] column layout
                    for qs in range(2):
                        nc.tensor.matmul(
                            l_ps[qs][:], ones_f32[:, 0:1],
                            l_acc[:, qs * 512 : (qs + 1) * 512],
                            start=True, stop=True,
                        )
                    lr = pp.tile([1, SH], f32, tag="lrow")
                    for qs in range(2):
                        nc.vector.tensor_copy(
                            lr[0:1, qs * 512 : (qs + 1) * 512], l_ps[qs][:]
                        )
                    nc.gpsimd.dma_start(out=l_d[b][:, :], in_=lr[:])
                    lcol = pp.tile([128, NCH], f32, tag="lcol")
                    nc.sync.dma_start(
                        out=lcol[:],
                        in_=l_d[b][0, :].rearrange("(c p) -> p c", p=128),
                    )
                    nc.vector.reciprocal(linv[b][:], lcol[:])

                attention(0, gath_k[0], gath_v[0])
                # branch 1 reloads q^T into the same slot (WAR handled by Tile)
                nc.scalar.dma_start(
                    out=qt[:],
                    in_=qTt_d[:, :].rearrange("(c p) x -> p c x", p=128),
                )
                attention(1, gath_k[1], gath_v[1])

            # ============== stage 3: output projection ==============
            with (
                tc.tile_pool(name="outp", bufs=1) as outp,
                tc.tile_pool(name="pso", bufs=2, space="PSUM") as pso,
            ):
                for oq in range(4):
                    wo = outp.tile([128, 16, 512], bf16, tag="wo", bufs=3)
                    nc.sync.dma_start(
                        out=wo[:],
                        in_=Wo16[:, oq * 512 : (oq + 1) * 512].rearrange(
                            "(c p) o -> p c o", p=128
                        ),
                    )
                    for qb in range(NCH):
                        psA = pso.tile([128, 512], f32, tag="poA")
                        psB = pso.tile([128, 512], f32, tag="poB")
                        for c in range(16):
                            lhsT = fusedT[:, c, qb * 128 : (qb + 1) * 128]
                            ps = psA if c < 8 else psB
                            nc.tensor.matmul(
                                ps[:], lhsT, wo[:, c, :],
                                start=(c % 8 == 0), stop=(c % 8 == 7),
                            )
                        o0 = oq * 512
                        # chunks 0..7 = attended_image = branch 1;
                        # chunks 8..15 = attended_tabular = branch 0
                        t1 = outp.tile([128, 512], f32, tag="t1")
                        nc.vector.scalar_tensor_tensor(
                            t1[:], psB[:], linv[0][:, qb : qb + 1],
                            bob_sb[:, o0 : o0 + 512], OP.mult, OP.add,
                        )
                        ost = outp.tile([128, 512], f32, tag="ost", bufs=4)
                        nc.vector.scalar_tensor_tensor(
                            ost[:], psA[:], linv[1][:, qb : qb + 1],
                            t1[:], OP.mult, OP.add,
                        )
                        nc.gpsimd.dma_start(
                            out=out[qb * 128 : (qb + 1) * 128, o0 : o0 + 512],
                            in_=ost[:],
                        )

    nc.compile()
    return nc


_CACHE: dict = {}


def kernel(
    image_features, tabular_features,
    Wqi, bqi, Wkt, bkt, Wvt, bvt,
    Wqt, bqt, Wki, bki, Wvi, bvi,
    Wo, bo,
) -> np.ndarray:
    if "nc" not in _CACHE:
        _CACHE["nc"] = build_nc()
    nc = _CACHE["nc"]

    img = np.asarray(image_features, np.float32)
    tab = np.asarray(tabular_features, np.float32)

    def bcol(b):
        return np.asarray(b, np.float32).reshape(NCH, 128).T

    shared = {
        "Wqi": np.asarray(Wqi, np.float32), "Wkt": np.asarray(Wkt, np.float32),
        "Wvt": np.asarray(Wvt, np.float32), "Wqt": np.asarray(Wqt, np.float32),
        "Wki": np.asarray(Wki, np.float32), "Wvi": np.asarray(Wvi, np.float32),
        "Wo16": np.asarray(Wo).astype(ml_dtypes.bfloat16),
        "bcol": np.ascontiguousarray(
            np.concatenate(
                [bcol(b) for b in (bqi, bkt, bqt, bki)], axis=1
            )
        ),
        "brow": np.ascontiguousarray(
            np.broadcast_to(
                np.concatenate(
                    [np.asarray(b).astype(ml_dtypes.bfloat16) for b in (bvt, bvi)]
                ).reshape(1, 2 * D),
                (128, 2 * D),
            )
        ),
        "bob": np.ascontiguousarray(
            np.broadcast_to(
                np.asarray(bo).astype(ml_dtypes.bfloat16).reshape(1, 2 * D),
                (128, 2 * D),
            )
        ),
    }
    in_maps = []
    for c in range(NCORES):
        m = dict(shared)
        m["xTi"] = np.ascontiguousarray(img[c * SH : (c + 1) * SH, :].T)
        m["xTt"] = np.ascontiguousarray(tab[c * SH : (c + 1) * SH, :].T)
        in_maps.append(m)

    trace = bool(int(os.environ.get("KERNEL_TRACE", "0")))
    res = run_bass_kernel_spmd(
        nc, in_maps, core_ids=list(range(NCORES)), trace=trace
    )
    _CACHE["last_result"] = res
    return np.concatenate([res.results[c]["out"] for c in range(NCORES)], axis=0)
